# revision 27
# baseline (speedup 1.0000x reference)
"""Multi-head self-attention (RoPE, eval-mode) Trainium2 Bass kernel.

Problem: B=2, T=2048, D=1024, H=16, d_head=64, fp32 I/O.

Sharding (8 cores): core c handles batch b=c//4 and the 4 heads
[4g, 4g+4) where g=c%4.  QKV/attention are head-local; the output
projection produces a per-core partial (contraction over this core's
256 head-dims) which the host sums across the 4 cores of each batch
and adds b_out.

v5: all-bf16 datapath (fp32 PSUM accumulation) + QKV/attention overlap.
  - bf16 weights get the PE's FastWeightLoad path (fp32r LDWEIGHTS is
    2-3x slower and partially serializes with matmuls), DVE elementwise
    runs at 2x, and input/output DMA bytes halve.
  - Only the attention-critical prefix of QKV runs serially (k for head
    pair 0, q block 0/1, v for the first half of T).  The rest of QKV
    (v tail, remaining q blocks, all of head-pair 1's q/k) is split into
    ~4-matmul background units drained one-per-tk inside the attention
    loop, where the ACT-bound inner loop leaves PE slack.  Background
    units accumulate in the outproj PSUM pool (transient slots) and
    combine into SBUF via DVE, so the exp pipeline's dedicated score
    slots are never stolen.
  - PSUM: sc 2x[128,1024] (score pipeline + serial-prefix chains),
    pv 1x[128,1024] (both heads side by side), po 2x[128,512]
    (outproj + v-chains + background units) = 8 banks.
  - DMAs are issued in priority order (w_qk, x quarter 0, w_v, rope
    tables, remaining x, w_o); the exp table load (~2.7us) is prepaid
    with a dummy exp while ACT is idle.

Per-core design notes:
  - q,k are computed feature-major (d_head on partitions, T on free) so
    scores^T tiles come straight from matmuls; 2 heads stacked per
    128-partition tile.  k is roped into per-head zero-padded K=128
    kpad tiles: row-group K=64 mms would be 2x denser but read as
    half-busy to HAM and interfere with LDWEIGHTS pull-ahead.
  - RoPE: rotate_half is a 32-partition block swap (SBUF->SBUF DMAs)
    with the sign folded into the host-provided sin table.
  - v is computed row-major [t, dv] and stored per head as [ones | v]
    128-wide stationary tiles, so each PV matmul yields the softmax
    denominators (partitions 0:64, replicated) and attn^T (64:128).
  - softmax skips max-subtraction (scores ~ N(0,1), exp safe) and
    normalizes after PV with one wide DVE reciprocal per block.
"""

import numpy as np
import ml_dtypes

B, T, D = 2, 2048, 1024
H = 16
DH = 64
NCORES = 8
P = 128

BF16 = ml_dtypes.bfloat16

_CACHE = {}


def _rope_tables_np():
    theta = 1.0 / (10000.0 ** (np.arange(0, DH, 2, dtype=np.float32) / DH))
    angles = np.outer(np.arange(T, dtype=np.float32), theta)  # (T, 32)
    angles = np.concatenate([angles, angles], axis=-1)  # (T, DH)
    cos = np.cos(angles).astype(np.float32)
    sin = np.sin(angles).astype(np.float32)
    cosT = np.ascontiguousarray(cos.T)  # (64, T)
    sinT = np.ascontiguousarray(sin.T)
    sinT_signed = np.concatenate([-sinT[0:32], sinT[32:64]], axis=0)
    cos2 = np.tile(cosT, (2, 1))  # (128, T)
    sin2 = np.tile(sinT_signed, (2, 1))
    return cos2.astype(BF16), sin2.astype(BF16)


def _build_module():
    import concourse.mybir as mybir
    import concourse.tile as tile
    from concourse import bacc

    f32 = mybir.dt.float32
    bf16 = mybir.dt.bfloat16

    nc = bacc.Bacc("TRN2", target_bir_lowering=False, debug=False)
    xT = nc.dram_tensor("xT", [4, P, 8, 512], bf16, kind="ExternalInput")
    w_qk = nc.dram_tensor("w_qk", [P, 8, 512], bf16, kind="ExternalInput")
    w_v = nc.dram_tensor("w_v", [P, 8, 256], bf16, kind="ExternalInput")
    w_o = nc.dram_tensor("w_o", [P, 2, 1024], bf16, kind="ExternalInput")
    cos2 = nc.dram_tensor("cos2", [P, T], bf16, kind="ExternalInput")
    sin2 = nc.dram_tensor("sin2", [P, T], bf16, kind="ExternalInput")
    out = nc.dram_tensor("out", [T, D], bf16, kind="ExternalOutput")

    Exp = mybir.ActivationFunctionType.Exp

    with tile.TileContext(nc) as tc:
        with (
            tc.tile_pool(name="persist", bufs=1) as persist,
            tc.tile_pool(name="attnsb", bufs=1) as apool,
            tc.tile_pool(name="expp", bufs=4) as epool,
            tc.tile_pool(name="norm", bufs=2) as npool,
            tc.tile_pool(name="xt", bufs=4) as xpool,
            tc.tile_pool(name="kst", bufs=1) as kpool,
            tc.tile_pool(name="rope", bufs=2) as rpool,
            tc.tile_pool(name="wop", bufs=1) as wpool,
            tc.tile_pool(name="ob", bufs=3) as opool,
            tc.tile_pool(name="sc_ps", bufs=2, space="PSUM") as scps,
            tc.tile_pool(name="pv_ps", bufs=1, space="PSUM") as pvps,
            tc.tile_pool(name="po_ps", bufs=2, space="PSUM") as pops,
        ):
            wqk_sb = [
                persist.tile([P, 4, 512], bf16, tag=f"wqk{i}", name=f"wqk{i}")
                for i in range(2)
            ]
            wv_sb = persist.tile([P, 8, 256], bf16)
            # q_q[hp][qtr]: roped q, two heads stacked, per T-quarter.
            q_q = [
                [
                    persist.tile([P, 512], bf16, tag=f"q{hp}_{q}", name=f"q{hp}_{q}")
                    for q in range(4)
                ]
                for hp in range(2)
            ]
            # kpad[hp][h][qtr]: roped k per head, zero-padded to K=128.
            kpad = [
                [
                    [
                        persist.tile(
                            [P, 512], bf16, tag=f"kp{hp}{h}_{q}", name=f"kp{hp}{h}_{q}"
                        )
                        for q in range(4)
                    ]
                    for h in range(2)
                ]
                for hp in range(2)
            ]
            # per (tk-tile, head): [ones | v] stationary 128x128
            vaug = persist.tile([P, 16, 4, P], bf16)
            attn_q = [
                [
                    apool.tile([P, 512], bf16, tag=f"at{hp}_{b}", name=f"at{hp}_{b}")
                    for b in range(4)
                ]
                for hp in range(2)
            ]
            cos_sb = kpool.tile([P, T], bf16)
            sin_sb = kpool.tile([P, T], bf16)
            scr = kpool.tile([64, 1], bf16, tag="scr", name="scr")
            kstack = [
                [
                    kpool.tile([P, 512], bf16, tag=f"ks{hp}_{q}", name=f"ks{hp}_{q}")
                    for q in range(4)
                ]
                for hp in range(2)
            ]
            wo_sb = wpool.tile([P, 2, 1024], bf16)

            for hp in range(2):
                for q in range(4):
                    nc.vector.memset(kpad[hp][0][q][64:128, :], 0.0)
                    nc.vector.memset(kpad[hp][1][q][0:64, :], 0.0)

            def fm_dst(cc, tq):
                return (q_q if cc in (0, 2) else kstack)[cc // 2][tq]

            def wqk_sl(dc, cc):
                return wqk_sb[dc // 4][:, dc % 4, cc * P : (cc + 1) * P]

            def fm_chain(xt, tq, cc):
                """One serial feature-major QKV chain (q or stacked k),
                through a score-pool slot (prefix only — ACT idle)."""
                ps = scps.tile([P, 1024], f32, tag="sc", name="fmps")
                for dc in range(8):
                    nc.tensor.matmul(
                        ps[:, 0:512],
                        lhsT=wqk_sl(dc, cc),
                        rhs=xt[dc // 4][:, dc % 4, :],
                        start=(dc == 0),
                        stop=(dc == 7),
                    )
                nc.scalar.copy(fm_dst(cc, tq)[:], ps[:, 0:512])

            def v_chain(xt, tq, t4):
                psv = pops.tile([P, 512], f32, tag="po", name="vps")
                for dc in range(8):
                    nc.tensor.matmul(
                        psv[:, 0:256],
                        lhsT=xt[dc // 4][:, dc % 4, t4 * P : (t4 + 1) * P],
                        rhs=wv_sb[:, dc, :],
                        start=(dc == 0),
                        stop=(dc == 7),
                    )
                tki = tq * 4 + t4
                nc.scalar.copy(
                    vaug[:, tki, :, 64:128],
                    psv[:, 0:256].rearrange("p (h e) -> p h e", e=64),
                )

            def rope_q(cc, qtr):
                """RoPE one T-quarter of one q/k tensor.  q is roped in
                place; stacked k is roped into the per-head zero-padded
                kpad tiles (two half-adds)."""
                base = fm_dst(cc, qtr)
                hs = slice(qtr * 512, (qtr + 1) * 512)
                rot = rpool.tile([P, 512], bf16, tag="rot", name="rot")
                for blk in range(4):
                    s = (blk ^ 1) * 32
                    eng = nc.sync if blk % 2 == 0 else nc.gpsimd
                    eng.dma_start(
                        rot[blk * 32 : (blk + 1) * 32, :], base[s : s + 32, :]
                    )
                t1 = rpool.tile([P, 512], bf16, tag="t1", name="t1")
                nc.vector.tensor_mul(t1[:], base[:], cos_sb[:, hs])
                nc.vector.tensor_mul(rot[:], rot[:], sin_sb[:, hs])
                if cc in (0, 2):
                    nc.vector.tensor_add(base[:], t1[:], rot[:])
                else:
                    hp = cc // 2
                    nc.vector.tensor_add(
                        kpad[hp][0][qtr][0:64, :], t1[0:64, :], rot[0:64, :]
                    )
                    nc.vector.tensor_add(
                        kpad[hp][1][qtr][64:128, :], t1[64:128, :], rot[64:128, :]
                    )

            # ---- background QKV units (drained inside the attention loop;
            # combines run on DVE since ACT is exp-bound there) -----------
            def bg_fm_pair_units(xts, cc, tqa, tqb):
                """Two background chains sharing each wqk stationary: one
                LDWEIGHTS feeds back-to-back matmuls for both quarters.
                Unit dc emits 2 matmuls; both pops slots are held for the
                9-unit span (no outproj in hp0, so no contention)."""
                st = {}

                def mk(dc):
                    def emit():
                        if dc == 0:
                            st["a"] = pops.tile([P, 512], f32, tag="po", name="bga")
                            st["b"] = pops.tile([P, 512], f32, tag="po", name="bgb")
                        for tq, key in ((tqa, "a"), (tqb, "b")):
                            nc.tensor.matmul(
                                st[key][:],
                                lhsT=wqk_sl(dc, cc),
                                rhs=xts[tq][dc // 4][:, dc % 4, :],
                                start=(dc == 0),
                                stop=(dc == 7),
                            )
                        if dc == 7:
                            nc.vector.tensor_copy(fm_dst(cc, tqa)[:], st["a"][:])
                            nc.vector.tensor_copy(fm_dst(cc, tqb)[:], st["b"][:])

                    return emit

                def ropes():
                    rope_q(cc, tqa)
                    rope_q(cc, tqb)

                return [mk(dc) for dc in range(8)] + [ropes]

            def bg_v_units(xt, tq, t4):
                tki = tq * 4 + t4
                vsl = vaug[:, tki, :, 64:128]

                def half(h0):
                    def emit():
                        ps = pops.tile([P, 512], f32, tag="po", name="bgvps")
                        for i in range(4):
                            dc = h0 + i
                            nc.tensor.matmul(
                                ps[:, 0:256],
                                lhsT=xt[dc // 4][:, dc % 4, t4 * P : (t4 + 1) * P],
                                rhs=wv_sb[:, dc, :],
                                start=(i == 0),
                                stop=(i == 3),
                            )
                        pr = ps[:, 0:256].rearrange("p (h e) -> p h e", e=64)
                        if h0 == 0:
                            nc.vector.tensor_copy(vsl, pr)
                        else:
                            nc.vector.tensor_add(vsl, vsl, pr)

                    return emit

                return [half(0), half(4)]

            # ---- DMA issue order = priority order ------------------------
            nc.scalar.dma_start(wqk_sb[0][:], w_qk[:, 0:4, :])
            nc.gpsimd.dma_start(wqk_sb[1][:], w_qk[:, 4:8, :])
            xts = []
            for tq in range(4):
                xtl = xpool.tile([P, 4, 512], bf16, tag="xtl", name="xtl")
                xth = xpool.tile([P, 4, 512], bf16, tag="xth", name="xth")
                xts.append((xtl, xth))
            nc.scalar.dma_start(wv_sb[:], w_v[:])
            nc.sync.dma_start(xts[0][0][:], xT[0, :, 0:4, :])
            nc.sync.dma_start(xts[0][1][:], xT[0, :, 4:8, :])
            nc.vector.memset(vaug[:, :, :, 0:64], 1.0)
            # prepay the ~2.7us exp table load while ACT is idle
            nc.scalar.activation(scr[:], vaug[0:64, 0, 0, 0:1], Exp, scale=0.125)
            # ~3.4us of dummy matmuls (on the ones region) during the DMA
            # wait flip HAM to 8/8 so the first real chains start at 2.4GHz
            warm = pops.tile([P, 512], f32, tag="po", name="warm")
            for i in range(32):
                nc.tensor.matmul(
                    warm[0:64, 0:128],
                    lhsT=vaug[:, 0, 0, 0:64],
                    rhs=vaug[:, 1, 0:2, 0:64],
                    start=(i == 0),
                    stop=(i == 31),
                )
            nc.scalar.dma_start(cos_sb[:], cos2[:])
            nc.scalar.dma_start(sin_sb[:], sin2[:])
            for tq in range(1, 4):
                nc.sync.dma_start(xts[tq][0][:], xT[tq, :, 0:4, :])
                nc.sync.dma_start(xts[tq][1][:], xT[tq, :, 4:8, :])
            nc.sync.dma_start(wo_sb[:], w_o[:])

            # ---- serial QKV prefix (DMA-paced) ---------------------------
            for tq in range(4):
                fm_chain(xts[tq], tq, 1)  # k hp0
                rope_q(1, tq)
                if tq < 2:
                    fm_chain(xts[tq], tq, 0)  # q hp0 blocks 0,1
                    rope_q(0, tq)
                    for t4 in range(4):
                        v_chain(xts[tq], tq, t4)  # v for tk 0..7

            # ---- background unit list (in deadline order) ----------------
            bg = []
            for tq in (2, 3):
                for t4 in range(4):
                    bg += bg_v_units(xts[tq], tq, t4)  # v tk 8..15
            bg += bg_fm_pair_units(xts, 0, 2, 3)  # q hp0 blocks 2,3
            bg += bg_fm_pair_units(xts, 3, 0, 1)  # k hp1
            bg += bg_fm_pair_units(xts, 3, 2, 3)
            bg += bg_fm_pair_units(xts, 2, 0, 1)  # q hp1
            bg += bg_fm_pair_units(xts, 2, 2, 3)

            # ---- attention + interleaved outproj + bg drain --------------
            def outproj_unit(b, tqc, via_pvps=False):
                # hp-outer: each attn stationary is loaded once and feeds
                # both d2 halves back-to-back.  When the current block's PV
                # accumulator lives in the po pool (odd hp1 blocks), the po
                # scratch borrows the idle pv-pool slot instead.
                row = b * 4 + tqc
                if via_pvps:
                    pot = pvps.tile([P, 1024], f32, tag="pv", name="pot")
                    po = [pot[:, 0:512], pot[:, 512:1024]]
                else:
                    po = [
                        pops.tile([P, 512], f32, tag="po", name="po")[:]
                        for _ in range(2)
                    ]
                for hp in range(2):
                    for d2 in range(2):
                        nc.tensor.matmul(
                            po[d2],
                            lhsT=attn_q[hp][b][:, tqc * P : (tqc + 1) * P],
                            rhs=wo_sb[:, hp, d2 * 512 : (d2 + 1) * 512],
                            start=(hp == 0),
                            stop=(hp == 1),
                        )
                for d2 in range(2):
                    ob = opool.tile([P, 512], bf16, tag="ob", name="ob")
                    nc.vector.tensor_copy(ob[:], po[d2])
                    seng = nc.sync if d2 == 0 else nc.gpsimd
                    seng.dma_start(
                        out[row * P : (row + 1) * P, d2 * 512 : (d2 + 1) * 512],
                        ob[:],
                    )

            # PV emission runs one tk behind the score/exp stream: at each
            # block boundary the next block's first score matmuls enter the
            # in-order PE queue BEFORE pv(15) (which waits on exp(15)), so
            # the exp pipeline never starves on the turn-around.
            state = {"prev": None}

            def flush_prev():
                p = state["prev"]
                if p is None:
                    return
                ppv, php, ptk, pex, pnorm = p
                for h in range(2):
                    nc.tensor.matmul(
                        ppv[h],
                        lhsT=vaug[:, ptk, php * 2 + h, :],
                        rhs=pex[:, h * 512 : (h + 1) * 512],
                        start=(ptk == 0),
                        stop=(ptk == 15),
                    )
                if pnorm is not None:
                    pnorm()
                state["prev"] = None

            for hp in range(2):
                for tq in range(4):  # tq blocks of 512
                    prev_b = tq - 1 if (hp == 1 and tq > 0) else None
                    # PV accumulator: hp1 alternates between the pv pool and
                    # the (bg-free in hp1) po pool so consecutive blocks
                    # never WAR on the same banks during normalize.
                    odd = hp == 1 and tq % 2 == 1
                    if odd:
                        pvh = [
                            pops.tile([P, 512], f32, tag="po", name=f"pvh{h}")
                            for h in range(2)
                        ]
                        pv = [pvh[0][:], pvh[1][:]]
                        den = [pvh[0][0:64, :], pvh[1][0:64, :]]
                        att = [pvh[0][64:128, :], pvh[1][64:128, :]]
                    else:
                        pvt = pvps.tile([P, 1024], f32, tag="pv", name="pv")
                        pv = [pvt[:, 0:512], pvt[:, 512:1024]]
                        den = [pvt[0:64, 0:1024]]
                        att = [pvt[64:128, 0:512], pvt[64:128, 512:1024]]

                    def make_norm(hp=hp, tq=tq, odd=odd, den=den, att=att):
                        def norm():
                            if odd:
                                for h in range(2):
                                    rc = npool.tile(
                                        [64, 512], f32, tag="rc2", name="rc2"
                                    )
                                    nc.vector.reciprocal_approx_fast(rc[:], den[h])
                                    nc.vector.tensor_mul(
                                        attn_q[hp][tq][h * 64 : (h + 1) * 64, :],
                                        att[h],
                                        rc[:],
                                    )
                            else:
                                rc = npool.tile([64, 1024], f32, tag="rc", name="rc")
                                nc.vector.reciprocal_approx_fast(rc[:], den[0])
                                for h in range(2):
                                    nc.vector.tensor_mul(
                                        attn_q[hp][tq][h * 64 : (h + 1) * 64, :],
                                        att[h],
                                        rc[:, h * 512 : (h + 1) * 512],
                                    )

                        return norm

                    norm = make_norm()
                    for tk in range(16):
                        if prev_b is not None and tk % 4 == 3:
                            outproj_unit(prev_b, tk // 4, via_pvps=odd)
                        elif bg:
                            bg.pop(0)()
                        sc = scps.tile([P, 1024], f32, tag="sc", name="sc")
                        ko = (tk % 4) * P
                        for h in range(2):
                            nc.tensor.matmul(
                                sc[:, h * 512 : (h + 1) * 512],
                                lhsT=kpad[hp][h][tk // 4][:, ko : ko + P],
                                rhs=q_q[hp][tq][:],
                                start=True,
                                stop=True,
                            )
                        ex = epool.tile([P, 1024], bf16, tag="e", name="e")
                        nc.scalar.activation(ex[:], sc[:], Exp, scale=0.125)
                        flush_prev()
                        state["prev"] = (pv, hp, tk, ex, norm if tk == 15 else None)
            flush_prev()
            for tqc in range(4):
                # alternate PSUM pools so consecutive trailing units don't
                # serialize on one slot through the DVE casts
                outproj_unit(3, tqc, via_pvps=(tqc % 2 == 0))

    nc.compile()
    return nc


def _get_module():
    if "nc" not in _CACHE:
        _CACHE["nc"] = _build_module()
    return _CACHE["nc"]


def make_in_maps(x, w_qkv, w_out):
    cos2, sin2 = _rope_tables_np()
    in_maps = []
    for c in range(NCORES):
        b, g = divmod(c, 4)
        q0 = 256 * g
        # column chunks: [q_hp0 | k_hp0 | q_hp1 | k_hp1]
        wqk_c = np.concatenate(
            [
                w_qkv[:, q0 : q0 + 128],
                w_qkv[:, 1024 + q0 : 1024 + q0 + 128],
                w_qkv[:, q0 + 128 : q0 + 256],
                w_qkv[:, 1024 + q0 + 128 : 1024 + q0 + 256],
            ],
            axis=1,
        )
        xt4 = np.ascontiguousarray(
            x[b].T.reshape(8, 128, 4, 512).transpose(2, 1, 0, 3)
        ).astype(BF16)
        wv_c = w_qkv[:, 2048 + q0 : 2048 + q0 + 256]
        in_maps.append(
            {
                "xT": xt4,
                "w_qk": np.ascontiguousarray(
                    wqk_c.reshape(8, 128, 512).transpose(1, 0, 2)
                ).astype(BF16),
                "w_v": np.ascontiguousarray(
                    wv_c.reshape(8, 128, 256).transpose(1, 0, 2)
                ).astype(BF16),
                "w_o": np.ascontiguousarray(
                    w_out[q0 : q0 + 256, :].reshape(2, 128, 1024).transpose(1, 0, 2)
                ).astype(BF16),
                "cos2": cos2,
                "sin2": sin2,
            }
        )
    return in_maps


def combine_outputs(results, b_out):
    out = np.empty((B, T, D), dtype=np.float32)
    for b in range(B):
        acc = results[4 * b]["out"].astype(np.float32)
        for c in range(4 * b + 1, 4 * b + 4):
            acc += results[c]["out"].astype(np.float32)
        out[b] = acc + b_out[None, :]
    return out


def kernel(x, w_qkv, w_out, b_out, _trace=False, _tag=[0]):
    from concourse import bass_utils

    nc = _get_module()
    in_maps = make_in_maps(
        np.asarray(x, dtype=np.float32),
        np.asarray(w_qkv, dtype=np.float32),
        np.asarray(w_out, dtype=np.float32),
    )
    res = bass_utils.run_bass_kernel_spmd(
        nc, in_maps, core_ids=list(range(NCORES)), trace=_trace
    )
    if _trace:
        _CACHE["last_result"] = res
    return combine_outputs(res.results, np.asarray(b_out, dtype=np.float32))


# revision 28
# speedup vs baseline: 1.1774x; 1.1774x over previous
"""Multi-head self-attention (RoPE, eval-mode) Trainium2 Bass kernel.

Problem: B=2, T=2048, D=1024, H=16, d_head=64, fp32 I/O.

Sharding (8 cores): core c handles batch b=c//4 and the 4 heads
[4g, 4g+4) where g=c%4.  QKV/attention are head-local; the output
projection produces a per-core partial (contraction over this core's
256 head-dims) which the host sums across the 4 cores of each batch
and adds b_out.

v5: all-bf16 datapath (fp32 PSUM accumulation) + QKV/attention overlap.
  - bf16 weights get the PE's FastWeightLoad path (fp32r LDWEIGHTS is
    2-3x slower and partially serializes with matmuls), DVE elementwise
    runs at 2x, and input/output DMA bytes halve.
  - Only the attention-critical prefix of QKV runs serially (k for head
    pair 0, q block 0/1, v for the first half of T).  The rest of QKV
    (v tail, remaining q blocks, all of head-pair 1's q/k) is split into
    ~4-matmul background units drained one-per-tk inside the attention
    loop, where the ACT-bound inner loop leaves PE slack.  Background
    units accumulate in the outproj PSUM pool (transient slots) and
    combine into SBUF via DVE, so the exp pipeline's dedicated score
    slots are never stolen.
  - PSUM: sc 2x[128,1024] (score pipeline + serial-prefix chains),
    pv 1x[128,1024] (both heads side by side), po 2x[128,512]
    (outproj + v-chains + background units) = 8 banks.
  - DMAs are issued in priority order (w_qk, x quarter 0, w_v, rope
    tables, remaining x, w_o); the exp table load (~2.7us) is prepaid
    with a dummy exp while ACT is idle.

Per-core design notes:
  - q,k are computed feature-major (d_head on partitions, T on free) so
    scores^T tiles come straight from matmuls; 2 heads stacked per
    128-partition tile.  k is roped into per-head zero-padded K=128
    kpad tiles: row-group K=64 mms would be 2x denser but read as
    half-busy to HAM and interfere with LDWEIGHTS pull-ahead.
  - RoPE: rotate_half is a 32-partition block swap (SBUF->SBUF DMAs)
    with the sign folded into the host-provided sin table.
  - v is computed row-major [t, dv] and stored per head as [ones | v]
    128-wide stationary tiles, so each PV matmul yields the softmax
    denominators (partitions 0:64, replicated) and attn^T (64:128).
  - softmax skips max-subtraction (scores ~ N(0,1), exp safe) and
    normalizes after PV with one wide DVE reciprocal per block.
"""

import numpy as np
import ml_dtypes

B, T, D = 2, 2048, 1024
H = 16
DH = 64
NCORES = 8
P = 128

BF16 = ml_dtypes.bfloat16

_CACHE = {}


def _rope_tables_np():
    theta = 1.0 / (10000.0 ** (np.arange(0, DH, 2, dtype=np.float32) / DH))
    angles = np.outer(np.arange(T, dtype=np.float32), theta)  # (T, 32)
    angles = np.concatenate([angles, angles], axis=-1)  # (T, DH)
    cos = np.cos(angles).astype(np.float32)
    sin = np.sin(angles).astype(np.float32)
    cosT = np.ascontiguousarray(cos.T)  # (64, T)
    sinT = np.ascontiguousarray(sin.T)
    sinT_signed = np.concatenate([-sinT[0:32], sinT[32:64]], axis=0)
    cos2 = np.tile(cosT, (2, 1))  # (128, T)
    sin2 = np.tile(sinT_signed, (2, 1))
    return cos2.astype(BF16), sin2.astype(BF16)


def _build_module():
    import concourse.mybir as mybir
    import concourse.tile as tile
    from concourse import bacc

    f32 = mybir.dt.float32
    bf16 = mybir.dt.bfloat16

    nc = bacc.Bacc("TRN2", target_bir_lowering=False, debug=False)
    xT = nc.dram_tensor("xT", [4, P, 8, 512], bf16, kind="ExternalInput")
    w_qk = nc.dram_tensor("w_qk", [P, 8, 512], bf16, kind="ExternalInput")
    w_v = nc.dram_tensor("w_v", [P, 8, 256], bf16, kind="ExternalInput")
    w_o = nc.dram_tensor("w_o", [P, 2, 1024], bf16, kind="ExternalInput")
    cos2 = nc.dram_tensor("cos2", [P, T], bf16, kind="ExternalInput")
    sin2 = nc.dram_tensor("sin2", [P, T], bf16, kind="ExternalInput")
    out = nc.dram_tensor("out", [T, D], bf16, kind="ExternalOutput")

    Exp = mybir.ActivationFunctionType.Exp

    with tile.TileContext(nc) as tc:
        with (
            tc.tile_pool(name="persist", bufs=1) as persist,
            tc.tile_pool(name="attnsb", bufs=1) as apool,
            tc.tile_pool(name="expp", bufs=4) as epool,
            tc.tile_pool(name="norm", bufs=2) as npool,
            tc.tile_pool(name="xt", bufs=4) as xpool,
            tc.tile_pool(name="kst", bufs=1) as kpool,
            tc.tile_pool(name="rope", bufs=2) as rpool,
            tc.tile_pool(name="wop", bufs=1) as wpool,
            tc.tile_pool(name="ob", bufs=3) as opool,
            tc.tile_pool(name="sc_ps", bufs=2, space="PSUM") as scps,
            tc.tile_pool(name="pv_ps", bufs=1, space="PSUM") as pvps,
            tc.tile_pool(name="po_ps", bufs=2, space="PSUM") as pops,
        ):
            wqk_sb = [
                persist.tile([P, 4, 512], bf16, tag=f"wqk{i}", name=f"wqk{i}")
                for i in range(2)
            ]
            wv_sb = persist.tile([P, 8, 256], bf16)
            # q_q[hp][qtr]: roped q, two heads stacked, per T-quarter.
            q_q = [
                [
                    persist.tile([P, 512], bf16, tag=f"q{hp}_{q}", name=f"q{hp}_{q}")
                    for q in range(4)
                ]
                for hp in range(2)
            ]
            # kpad[hp][h][qtr]: roped k per head, zero-padded to K=128.
            kpad = [
                [
                    [
                        persist.tile(
                            [P, 512], bf16, tag=f"kp{hp}{h}_{q}", name=f"kp{hp}{h}_{q}"
                        )
                        for q in range(4)
                    ]
                    for h in range(2)
                ]
                for hp in range(2)
            ]
            # per (tk-tile, head): [ones | v] stationary 128x128
            vaug = persist.tile([P, 16, 4, P], bf16)
            attn_q = [
                [
                    apool.tile([P, 512], bf16, tag=f"at{hp}_{b}", name=f"at{hp}_{b}")
                    for b in range(4)
                ]
                for hp in range(2)
            ]
            cos_sb = kpool.tile([P, T], bf16)
            sin_sb = kpool.tile([P, T], bf16)
            scr = kpool.tile([64, 1], bf16, tag="scr", name="scr")
            kstack = [
                [
                    kpool.tile([P, 512], bf16, tag=f"ks{hp}_{q}", name=f"ks{hp}_{q}")
                    for q in range(4)
                ]
                for hp in range(2)
            ]
            wo_sb = wpool.tile([P, 2, 1024], bf16)

            for hp in range(2):
                for q in range(4):
                    nc.vector.memset(kpad[hp][0][q][64:128, :], 0.0)
                    nc.vector.memset(kpad[hp][1][q][0:64, :], 0.0)

            def fm_dst(cc, tq):
                return (q_q if cc in (0, 2) else kstack)[cc // 2][tq]

            def wqk_sl(dc, cc):
                return wqk_sb[dc // 4][:, dc % 4, cc * P : (cc + 1) * P]

            def fm_chain(xt, tq, cc):
                """One serial feature-major QKV chain (q or stacked k),
                through a score-pool slot (prefix only — ACT idle)."""
                ps = scps.tile([P, 1024], f32, tag="sc", name="fmps")
                for dc in range(8):
                    nc.tensor.matmul(
                        ps[:, 0:512],
                        lhsT=wqk_sl(dc, cc),
                        rhs=xt[dc // 4][:, dc % 4, :],
                        start=(dc == 0),
                        stop=(dc == 7),
                    )
                nc.scalar.copy(fm_dst(cc, tq)[:], ps[:, 0:512])

            def v_chain(xt, tq, t4):
                psv = pops.tile([P, 512], f32, tag="po", name="vps")
                for dc in range(8):
                    nc.tensor.matmul(
                        psv[:, 0:256],
                        lhsT=xt[dc // 4][:, dc % 4, t4 * P : (t4 + 1) * P],
                        rhs=wv_sb[:, dc, :],
                        start=(dc == 0),
                        stop=(dc == 7),
                    )
                tki = tq * 4 + t4
                nc.scalar.copy(
                    vaug[:, tki, :, 64:128],
                    psv[:, 0:256].rearrange("p (h e) -> p h e", e=64),
                )

            def rope_q(cc, qtr):
                """RoPE one T-quarter of one q/k tensor.  q is roped in
                place; stacked k is roped into the per-head zero-padded
                kpad tiles (two half-adds)."""
                base = fm_dst(cc, qtr)
                hs = slice(qtr * 512, (qtr + 1) * 512)
                rot = rpool.tile([P, 512], bf16, tag="rot", name="rot")
                for blk in range(4):
                    s = (blk ^ 1) * 32
                    eng = nc.sync if blk % 2 == 0 else nc.gpsimd
                    eng.dma_start(
                        rot[blk * 32 : (blk + 1) * 32, :], base[s : s + 32, :]
                    )
                t1 = rpool.tile([P, 512], bf16, tag="t1", name="t1")
                nc.vector.tensor_mul(t1[:], base[:], cos_sb[:, hs])
                nc.vector.tensor_mul(rot[:], rot[:], sin_sb[:, hs])
                if cc in (0, 2):
                    nc.vector.tensor_add(base[:], t1[:], rot[:])
                else:
                    hp = cc // 2
                    nc.vector.tensor_add(
                        kpad[hp][0][qtr][0:64, :], t1[0:64, :], rot[0:64, :]
                    )
                    nc.vector.tensor_add(
                        kpad[hp][1][qtr][64:128, :], t1[64:128, :], rot[64:128, :]
                    )

            # ---- background QKV units (drained inside the attention loop;
            # combines run on DVE since ACT is exp-bound there) -----------
            def bg_fm_pair_units(xts, cc, tqa, tqb):
                """Two background chains sharing each wqk stationary: one
                LDWEIGHTS feeds back-to-back matmuls for both quarters.
                Unit dc emits 2 matmuls; both pops slots are held for the
                9-unit span (no outproj in hp0, so no contention)."""
                st = {}

                def mk(dc):
                    def emit():
                        if dc == 0:
                            st["a"] = pops.tile([P, 512], f32, tag="po", name="bga")
                            st["b"] = pops.tile([P, 512], f32, tag="po", name="bgb")
                        for tq, key in ((tqa, "a"), (tqb, "b")):
                            nc.tensor.matmul(
                                st[key][:],
                                lhsT=wqk_sl(dc, cc),
                                rhs=xts[tq][dc // 4][:, dc % 4, :],
                                start=(dc == 0),
                                stop=(dc == 7),
                            )
                        if dc == 7:
                            nc.vector.tensor_copy(fm_dst(cc, tqa)[:], st["a"][:])
                            nc.vector.tensor_copy(fm_dst(cc, tqb)[:], st["b"][:])

                    return emit

                def ropes():
                    rope_q(cc, tqa)
                    rope_q(cc, tqb)

                return [mk(dc) for dc in range(8)] + [ropes]

            def bg_v_units(xt, tq, t4):
                tki = tq * 4 + t4
                vsl = vaug[:, tki, :, 64:128]

                def half(h0):
                    def emit():
                        ps = pops.tile([P, 512], f32, tag="po", name="bgvps")
                        for i in range(4):
                            dc = h0 + i
                            nc.tensor.matmul(
                                ps[:, 0:256],
                                lhsT=xt[dc // 4][:, dc % 4, t4 * P : (t4 + 1) * P],
                                rhs=wv_sb[:, dc, :],
                                start=(i == 0),
                                stop=(i == 3),
                            )
                        pr = ps[:, 0:256].rearrange("p (h e) -> p h e", e=64)
                        if h0 == 0:
                            nc.vector.tensor_copy(vsl, pr)
                        else:
                            nc.vector.tensor_add(vsl, vsl, pr)

                    return emit

                return [half(0), half(4)]

            # ---- DMA issue order = priority order ------------------------
            nc.scalar.dma_start(wqk_sb[0][:], w_qk[:, 0:4, :])
            nc.gpsimd.dma_start(wqk_sb[1][:], w_qk[:, 4:8, :])
            xts = []
            for tq in range(4):
                xtl = xpool.tile([P, 4, 512], bf16, tag="xtl", name="xtl")
                xth = xpool.tile([P, 4, 512], bf16, tag="xth", name="xth")
                xts.append((xtl, xth))
            nc.scalar.dma_start(wv_sb[:], w_v[:])
            nc.sync.dma_start(xts[0][0][:], xT[0, :, 0:4, :])
            nc.sync.dma_start(xts[0][1][:], xT[0, :, 4:8, :])
            nc.vector.memset(vaug[:, :, :, 0:64], 1.0)
            # prepay the ~2.7us exp table load while ACT is idle
            nc.scalar.activation(scr[:], vaug[0:64, 0, 0, 0:1], Exp, scale=0.125)
            nc.scalar.dma_start(cos_sb[:], cos2[:])
            nc.scalar.dma_start(sin_sb[:], sin2[:])
            for tq in range(1, 4):
                nc.sync.dma_start(xts[tq][0][:], xT[tq, :, 0:4, :])
                nc.sync.dma_start(xts[tq][1][:], xT[tq, :, 4:8, :])
            nc.sync.dma_start(wo_sb[:], w_o[:])

            # ---- serial QKV prefix (DMA-paced) ---------------------------
            for tq in range(4):
                fm_chain(xts[tq], tq, 1)  # k hp0
                rope_q(1, tq)
                if tq < 2:
                    fm_chain(xts[tq], tq, 0)  # q hp0 blocks 0,1
                    rope_q(0, tq)
                    for t4 in range(4):
                        v_chain(xts[tq], tq, t4)  # v for tk 0..7

            # ---- background unit list (in deadline order) ----------------
            bg = []
            for tq in (2, 3):
                for t4 in range(4):
                    bg += bg_v_units(xts[tq], tq, t4)  # v tk 8..15
            bg += bg_fm_pair_units(xts, 0, 2, 3)  # q hp0 blocks 2,3
            bg += bg_fm_pair_units(xts, 3, 0, 1)  # k hp1
            bg += bg_fm_pair_units(xts, 3, 2, 3)
            bg += bg_fm_pair_units(xts, 2, 0, 1)  # q hp1
            bg += bg_fm_pair_units(xts, 2, 2, 3)

            # ---- attention + interleaved outproj + bg drain --------------
            def outproj_unit(b, tqc, via_pvps=False):
                # hp-outer: each attn stationary is loaded once and feeds
                # both d2 halves back-to-back.  When the current block's PV
                # accumulator lives in the po pool (odd hp1 blocks), the po
                # scratch borrows the idle pv-pool slot instead.
                row = b * 4 + tqc
                if via_pvps:
                    pot = pvps.tile([P, 1024], f32, tag="pv", name="pot")
                    po = [pot[:, 0:512], pot[:, 512:1024]]
                else:
                    po = [
                        pops.tile([P, 512], f32, tag="po", name="po")[:]
                        for _ in range(2)
                    ]
                for hp in range(2):
                    for d2 in range(2):
                        nc.tensor.matmul(
                            po[d2],
                            lhsT=attn_q[hp][b][:, tqc * P : (tqc + 1) * P],
                            rhs=wo_sb[:, hp, d2 * 512 : (d2 + 1) * 512],
                            start=(hp == 0),
                            stop=(hp == 1),
                        )
                for d2 in range(2):
                    ob = opool.tile([P, 512], bf16, tag="ob", name="ob")
                    nc.vector.tensor_copy(ob[:], po[d2])
                    seng = nc.sync if d2 == 0 else nc.gpsimd
                    seng.dma_start(
                        out[row * P : (row + 1) * P, d2 * 512 : (d2 + 1) * 512],
                        ob[:],
                    )

            for hp in range(2):
                for tq in range(4):  # tq blocks of 512
                    prev_b = tq - 1 if (hp == 1 and tq > 0) else None
                    # PV accumulator: hp1 alternates between the pv pool and
                    # the (bg-free in hp1) po pool so consecutive blocks
                    # never WAR on the same banks during normalize.
                    odd = hp == 1 and tq % 2 == 1
                    if odd:
                        pvh = [
                            pops.tile([P, 512], f32, tag="po", name=f"pvh{h}")
                            for h in range(2)
                        ]
                        pv = [pvh[0][:], pvh[1][:]]
                        den = [pvh[0][0:64, :], pvh[1][0:64, :]]
                        att = [pvh[0][64:128, :], pvh[1][64:128, :]]
                    else:
                        pvt = pvps.tile([P, 1024], f32, tag="pv", name="pv")
                        pv = [pvt[:, 0:512], pvt[:, 512:1024]]
                        den = [pvt[0:64, 0:1024]]
                        att = [pvt[64:128, 0:512], pvt[64:128, 512:1024]]
                    for tk in range(16):
                        if prev_b is not None and tk % 4 == 3:
                            outproj_unit(prev_b, tk // 4, via_pvps=odd)
                        elif bg:
                            bg.pop(0)()
                        sc = scps.tile([P, 1024], f32, tag="sc", name="sc")
                        ko = (tk % 4) * P
                        for h in range(2):
                            nc.tensor.matmul(
                                sc[:, h * 512 : (h + 1) * 512],
                                lhsT=kpad[hp][h][tk // 4][:, ko : ko + P],
                                rhs=q_q[hp][tq][:],
                                start=True,
                                stop=True,
                            )
                        ex = epool.tile([P, 1024], bf16, tag="e", name="e")
                        nc.scalar.activation(ex[:], sc[:], Exp, scale=0.125)
                        for h in range(2):
                            nc.tensor.matmul(
                                pv[h],
                                lhsT=vaug[:, tk, hp * 2 + h, :],
                                rhs=ex[:, h * 512 : (h + 1) * 512],
                                start=(tk == 0),
                                stop=(tk == 15),
                            )
                    if odd:
                        for h in range(2):
                            rc = npool.tile([64, 512], f32, tag="rc2", name="rc2")
                            nc.vector.reciprocal_approx_fast(rc[:], den[h])
                            nc.vector.tensor_mul(
                                attn_q[hp][tq][h * 64 : (h + 1) * 64, :],
                                att[h],
                                rc[:],
                            )
                    else:
                        # one wide reciprocal covers both heads' denominators
                        rc = npool.tile([64, 1024], f32, tag="rc", name="rc")
                        nc.vector.reciprocal_approx_fast(rc[:], den[0])
                        for h in range(2):
                            nc.vector.tensor_mul(
                                attn_q[hp][tq][h * 64 : (h + 1) * 64, :],
                                att[h],
                                rc[:, h * 512 : (h + 1) * 512],
                            )
            for tqc in range(4):
                outproj_unit(3, tqc, via_pvps=True)

    nc.compile()
    return nc


def _get_module():
    if "nc" not in _CACHE:
        _CACHE["nc"] = _build_module()
    return _CACHE["nc"]


def make_in_maps(x, w_qkv, w_out):
    cos2, sin2 = _rope_tables_np()
    in_maps = []
    for c in range(NCORES):
        b, g = divmod(c, 4)
        q0 = 256 * g
        # column chunks: [q_hp0 | k_hp0 | q_hp1 | k_hp1]
        wqk_c = np.concatenate(
            [
                w_qkv[:, q0 : q0 + 128],
                w_qkv[:, 1024 + q0 : 1024 + q0 + 128],
                w_qkv[:, q0 + 128 : q0 + 256],
                w_qkv[:, 1024 + q0 + 128 : 1024 + q0 + 256],
            ],
            axis=1,
        )
        xt4 = np.ascontiguousarray(
            x[b].T.reshape(8, 128, 4, 512).transpose(2, 1, 0, 3)
        ).astype(BF16)
        wv_c = w_qkv[:, 2048 + q0 : 2048 + q0 + 256]
        in_maps.append(
            {
                "xT": xt4,
                "w_qk": np.ascontiguousarray(
                    wqk_c.reshape(8, 128, 512).transpose(1, 0, 2)
                ).astype(BF16),
                "w_v": np.ascontiguousarray(
                    wv_c.reshape(8, 128, 256).transpose(1, 0, 2)
                ).astype(BF16),
                "w_o": np.ascontiguousarray(
                    w_out[q0 : q0 + 256, :].reshape(2, 128, 1024).transpose(1, 0, 2)
                ).astype(BF16),
                "cos2": cos2,
                "sin2": sin2,
            }
        )
    return in_maps


def combine_outputs(results, b_out):
    out = np.empty((B, T, D), dtype=np.float32)
    for b in range(B):
        acc = results[4 * b]["out"].astype(np.float32)
        for c in range(4 * b + 1, 4 * b + 4):
            acc += results[c]["out"].astype(np.float32)
        out[b] = acc + b_out[None, :]
    return out


def kernel(x, w_qkv, w_out, b_out, _trace=False, _tag=[0]):
    from concourse import bass_utils

    nc = _get_module()
    in_maps = make_in_maps(
        np.asarray(x, dtype=np.float32),
        np.asarray(w_qkv, dtype=np.float32),
        np.asarray(w_out, dtype=np.float32),
    )
    res = bass_utils.run_bass_kernel_spmd(
        nc, in_maps, core_ids=list(range(NCORES)), trace=_trace
    )
    if _trace:
        _CACHE["last_result"] = res
    return combine_outputs(res.results, np.asarray(b_out, dtype=np.float32))
